# revision 11
# baseline (speedup 1.0000x reference)
"""Trainium2 Bass kernel for DecoderCrossAttention (sparse attention).

Math (per query n in batch b, H=8 heads, DH=64):
  q = x @ W_q;  k_s/v_s from per-query spatial_context (K_SP=32 keys);
  k_g/v_g from per-batch global_context (L_G=256);  softmax over 32+256
  scores; out = (attn_s V_s + attn_g V_g) @ W_out + b_out.

Key restructuring: k_s = sp @ W_ks is only ever consumed through
scores_s[n,h,k] = q[n,h] . k_s[n,k,h], so we fold W_ks into the query
side:  qs~[n,h,:] = sum_dh q[n,h,dh] * W_ks[:, h*64+dh]  (one 512-vector
per (n,h)), and scores_s[n,h,k] = qs~[n,h,:] . sp[n,k,:].  Likewise the
spatial V side uses ctx[n,h,:] = sum_k attn_s[n,h,k] sp[n,k,:] followed by
inner_s[n,h*64:+64] = ctx[n,h,:] @ W_vs[:, h*64:+64].  This avoids the
two (16384,512)@(512,512) matmuls per core entirely (~11x FLOP cut) and
makes the kernel HBM-bound on streaming spatial_context.

Sharding: (B=4, N=1024) -> 4096 queries split into 8 shards of 512; each
core gets its half-batch's queries + their spatial context + that batch's
global context; weights replicated.  Within a core: 4 chunks of 128
queries, each chunk processed as 8 groups of 16 queries.  Per 16-query
group, scores are computed as one dense (128x512) cross matmul
([8h x 16q] x [16q x 32k]) with off-diagonal pairs masked to -inf before
softmax, which also leaves the full softmaxed cross matrix directly
usable as the ctx matmul stationary operand.
"""

import sys

for _p in ("/opt/trn_rl_repo",):
    if _p not in sys.path:
        sys.path.insert(0, _p)

import numpy as np
import ml_dtypes

import concourse.bass as bass
import concourse.mybir as mybir
import concourse.tile as tile
from concourse.bass import ts, ds
from concourse.bass_utils import run_bass_kernel_spmd
from concourse.masks import make_identity

BF16 = ml_dtypes.bfloat16

# Problem shapes (hardcoded per contract)
B, N, K_SP, L_G = 4, 1024, 32, 256
D = 512          # DQ = DS = DG = DOUT
H, DH = 8, 64
INNER = H * DH   # 512
NC = 8           # cores
NQ = (B * N) // NC       # 512 queries per core
CHUNK = 128              # queries per chunk
GQ = 16                  # queries per group
GROUPS_PER_CHUNK = CHUNK // GQ   # 8
CHUNKS = NQ // CHUNK             # 4
GROUPS = NQ // GQ                # 32
GROW = GQ * K_SP                 # 512 sp rows per group
SW = GQ * K_SP                   # 512 spatial cross-score cols
TW = SW + L_G                    # 768 total score cols
SCALE = DH ** -0.5

_F32 = mybir.dt.float32
_BF16 = mybir.dt.bfloat16

_NEG = -30000.0


DEBUG_TAPS = False
SPLIT_WAITS = True


def _build_module():
    nc = bass.Bass("TRN2", target_bir_lowering=False, debug=False, num_devices=NC)

    def din(name, shape, dt=_BF16):
        return nc.dram_tensor(name, list(shape), dt, kind="ExternalInput").ap()

    xT = din("xT", (128, 4, NQ))                 # x_shard.T, D-major chunks
    spn = din("spn", (GROUPS, 128, 4, D))        # sp rows, [group][p][kc][D]
    spt = din("spt", (GROUPS, 128, 4, GROW))     # sp.T, [group][p][dm][row]
    gT = din("gT", (128, 4, L_G))                # g_shard.T
    wq = din("wq", (128, 4, INNER))              # W_q (D-major chunks)
    wksT = din("wksT", (128, 4, D))              # W_ks.T (INNER-major chunks)
    wvs = din("wvs", (128, 4, INNER))            # W_vs
    wkg = din("wkg", (128, 4, INNER))            # W_kg
    wvg = din("wvg", (128, 4, INNER))            # W_vg
    wout = din("wout", (128, 4, D))              # W_out (INNER-major chunks)
    maskc = din("maskc", (128, TW), _F32)        # -inf off-diag spatial mask
    biasc = din("biasc", (1, D), _F32)           # b_out
    out = nc.dram_tensor("out", [NQ, D], _F32, kind="ExternalOutput").ap()
    taps = None
    if DEBUG_TAPS:
        taps = {
            "qT": nc.dram_tensor("tap_qT", [128, 4, NQ], _BF16, kind="ExternalOutput").ap(),
            "qsT": nc.dram_tensor("tap_qsT", [128, 4, GROUPS, 128], _BF16, kind="ExternalOutput").ap(),
            "sc": nc.dram_tensor("tap_sc", [GROUPS, 128, TW], _F32, kind="ExternalOutput").ap(),
            "atb": nc.dram_tensor("tap_atb", [GROUPS, 128, TW], _BF16, kind="ExternalOutput").ap(),
            "ctxT": nc.dram_tensor("tap_ctxT", [128, 4, H, NQ], _BF16, kind="ExternalOutput").ap(),
            "agT": nc.dram_tensor("tap_agT", [128, 2, H, NQ], _BF16, kind="ExternalOutput").ap(),
            "inner": nc.dram_tensor("tap_inner", [CHUNKS, 128, INNER], _BF16, kind="ExternalOutput").ap(),
        }

    with tile.TileContext(nc) as tc:
        _emit(nc, tc, xT, spn, spt, gT, wq, wksT, wvs, wkg, wvg, wout,
              maskc, biasc, out, taps)

    # this walrus build accepts only one sync wait per instruction
    if SPLIT_WAITS:
        _split_excess_waits(nc, max_waits=1)
    return nc


def _emit(nc, tc, xT_d, spn_d, spt_d, gT_d, wq_d, wksT_d, wvs_d, wkg_d,
          wvg_d, wout_d, mask_d, bias_d, out_d, taps=None):

    def tap(name, sbuf_ap, dram_slice=None):
        if taps is None or name not in taps:
            return
        dst = taps[name] if dram_slice is None else dram_slice
        tmp = tc  # noqa
        nc.gpsimd.dma_start(out=dst, in_=sbuf_ap)

    import contextlib
    ctx = contextlib.ExitStack()
    with ctx:
        singles = ctx.enter_context(tc.tile_pool(name="singles", bufs=1))
        sp_pool = ctx.enter_context(tc.tile_pool(name="sp", bufs=3))
        work = ctx.enter_context(tc.tile_pool(name="work", bufs=2))
        ps_big = ctx.enter_context(tc.tile_pool(name="ps_big", bufs=2, space="PSUM"))
        ps_mid = ctx.enter_context(tc.tile_pool(name="ps_mid", bufs=2, space="PSUM"))
        ps_t = ctx.enter_context(tc.tile_pool(name="ps_t", bufs=2, space="PSUM"))

        # ---- constants ----
        def load_const(dram, shape, dt=_BF16, tag=None):
            t = singles.tile(list(shape), dt, tag=tag)
            nc.sync.dma_start(out=t, in_=dram)
            return t

        wq = load_const(wq_d, (128, 4, INNER), tag="wq")
        wksT = load_const(wksT_d, (128, 4, D), tag="wksT")
        wvs = load_const(wvs_d, (128, 4, INNER), tag="wvs")
        wkg = load_const(wkg_d, (128, 4, INNER), tag="wkg")
        wvg = load_const(wvg_d, (128, 4, INNER), tag="wvg")
        wout = load_const(wout_d, (128, 4, D), tag="wout")
        xT = load_const(xT_d, (128, 4, NQ), tag="xT")
        gT = load_const(gT_d, (128, 4, L_G), tag="gT")
        mask = load_const(mask_d, (128, TW), _F32, tag="mask")

        bias = singles.tile([128, D], _F32, tag="bias")
        nc.sync.dma_start(
            out=bias,
            in_=bass.AP(tensor=bias_d.tensor, offset=bias_d.offset,
                        ap=[[0, 128]] + bias_d.ap[1:]))

        ident = singles.tile([128, 128], _BF16, tag="ident")
        make_identity(nc, ident)

        def load_group(dram, g, tag):
            t = sp_pool.tile([128, 4, 512], _BF16, tag=tag)
            nc.sync.dma_start(out=t, in_=dram[g])
            return t

        # DMA loads with proper layout already; weights reshaped on host so
        # [p, ko, m] indexing == W[ko*128+p, m].

        # ---- stage A: qT, k_gT, v_g, qsT ----
        # qT[(h,dh), n] = SCALE * sum_D W_q[D,(h,dh)] x[n, D]
        qT = singles.tile([128, 4, NQ], _BF16, tag="qT")
        for mi in range(4):
            ps = ps_mid.tile([128, NQ], _F32, tag="psm")
            for ko in range(4):
                nc.tensor.matmul(ps, wq[:, ko, ts(mi, 128)], xT[:, ko, :],
                                 start=(ko == 0), stop=(ko == 3))
            nc.scalar.activation(qT[:, mi, :], ps,
                                 mybir.ActivationFunctionType.Copy,
                                 bias=0.0, scale=SCALE)
            if taps is not None:
                nc.gpsimd.dma_start(out=taps["qT"][:, mi, :], in_=qT[:, mi, :])

        # k_gT[(h,dh), l] (unscaled; scale lives in qT)
        k_gT = singles.tile([128, 4, L_G], _BF16, tag="k_gT")
        for mi in range(4):
            ps = ps_mid.tile([128, L_G], _F32, tag="psm")
            for ko in range(4):
                nc.tensor.matmul(ps, wkg[:, ko, ts(mi, 128)], gT[:, ko, :],
                                 start=(ko == 0), stop=(ko == 3))
            nc.vector.tensor_copy(k_gT[:, mi, :], ps)

        # v_g[l, (h,dh)]
        v_g = singles.tile([128, 2, INNER], _BF16, tag="v_g")
        for li in range(2):
            ps = ps_mid.tile([128, INNER], _F32, tag="psm")
            for ko in range(4):
                nc.tensor.matmul(ps, gT[:, ko, ts(li, 128)], wvg[:, ko, :],
                                 start=(ko == 0), stop=(ko == 3))
            nc.vector.tensor_copy(v_g[:, li, :], ps)

        # qsT[(dm,p), dm, G, (h,i)] = qs~[n=G*16+i, h, dm*128+p]
        # group-contiguous columns so the scores lhsT slice is 1 free dim
        qsT = singles.tile([128, 4, GROUPS, 128], _BF16, tag="qsT")
        for h in range(8):
            po = (h % 2) * 64
            io = h // 2
            for mi in range(4):
                ps = ps_mid.tile([128, NQ], _F32, tag="psm")
                nc.tensor.matmul(ps, wksT[po:po + 64, io, ts(mi, 128)],
                                 qT[po:po + 64, io, :], start=True, stop=True)
                nc.vector.tensor_copy(
                    qsT[:, mi, :, ds(h * GQ, GQ)],
                    ps.rearrange("p (g i) -> p g i", i=GQ))
                if taps is not None:
                    nc.gpsimd.dma_start(
                        out=taps["qsT"][:, mi, :, ds(h * GQ, GQ)],
                        in_=qsT[:, mi, :, ds(h * GQ, GQ)])

        # attn_gT_full[(lk,p), lk, h, n] built per group; read per chunk
        agT = singles.tile([128, 2, H, NQ], _BF16, tag="agT")
        # ctxT[(dm,p), dm, h, n]
        ctxT = singles.tile([128, 4, H, NQ], _BF16, tag="ctxT")

        for c in range(CHUNKS):
            for g in range(GROUPS_PER_CHUNK):
                G = c * GROUPS_PER_CHUNK + g
                col0 = c * CHUNK + g * GQ

                spn = load_group(spn_d, G, "spn")  # [p, kc, D] rows natural
                spt = load_group(spt_d, G, "spt")  # [p, dm, row] sp.T

                # scores: psum [128=(8h,16i), 768]
                ps_sc = ps_big.tile([128, TW], _F32, tag="psb")
                for dm in range(4):
                    nc.tensor.matmul(ps_sc[:, 0:SW],
                                     qsT[:, dm, G, :],
                                     spt[:, dm, :],
                                     start=(dm == 0), stop=(dm == 3))
                # global scores via head-masked qT copy
                mq = work.tile([128, 4, 128], _BF16, tag="mq")
                nc.gpsimd.memset(mq, 0.0)
                for h in range(8):
                    po = (h % 2) * 64
                    nc.gpsimd.tensor_copy(
                        mq[po:po + 64, h // 2, ts(h, GQ)],
                        qT[po:po + 64, h // 2, ds(col0, GQ)])
                for io in range(4):
                    nc.tensor.matmul(ps_sc[:, SW:TW], mq[:, io, :],
                                     k_gT[:, io, :],
                                     start=(io == 0), stop=(io == 3))

                # softmax over 768 (masked off-diag -> exp 0)
                sc = work.tile([128, TW], _F32, tag="sc")
                nc.vector.tensor_add(sc, ps_sc, mask)
                nmx = work.tile([128, 1], _F32, tag="nmx")
                nc.vector.reduce_max(nmx, sc, axis=mybir.AxisListType.X, negate=True)
                ex = work.tile([128, TW], _F32, tag="ex")
                sm = work.tile([128, 1], _F32, tag="sm")
                nc.scalar.activation(ex, sc, mybir.ActivationFunctionType.Exp,
                                     bias=nmx, scale=1.0, accum_out=sm)
                rc = work.tile([128, 1], _F32, tag="rc")
                nc.vector.reciprocal(rc, sm)
                atb = work.tile([128, TW], _BF16, tag="atb")
                nc.vector.tensor_scalar_mul(atb, ex, rc)
                if taps is not None:
                    nc.gpsimd.dma_start(out=taps["sc"][G], in_=sc)
                    nc.gpsimd.dma_start(out=taps["atb"][G], in_=atb)

                # transpose attn: spatial 4 blocks -> asT, global 2 -> agT
                asT = work.tile([128, 4, 128], _BF16, tag="asT")
                for blk in range(4):
                    pt = ps_t.tile([128, 128], _BF16, tag="pst")
                    nc.tensor.transpose(pt, atb[:, ts(blk, 128)], ident)
                    nc.scalar.copy(asT[:, blk, :], pt)
                for lk in range(2):
                    pt = ps_t.tile([128, 128], _BF16, tag="pst")
                    nc.tensor.transpose(pt, atb[:, ds(SW + lk * 128, 128)], ident)
                    nc.scalar.copy(agT[:, lk, :, ds(col0, GQ)],
                                   pt.rearrange("p (h i) -> p h i", h=H))

                # ctx[(h,i), D] = attn_sT.T @ sp_rows
                ps_c = ps_mid.tile([128, D], _F32, tag="psm")
                for kc in range(4):
                    nc.tensor.matmul(ps_c, asT[:, kc, :], spn[:, kc, :],
                                     start=(kc == 0), stop=(kc == 3))
                cxb = work.tile([128, D], _BF16, tag="cxb")
                nc.vector.tensor_copy(cxb, ps_c)
                for dm in range(4):
                    pt = ps_t.tile([128, 128], _BF16, tag="pst")
                    nc.tensor.transpose(pt, cxb[:, ts(dm, 128)], ident)
                    nc.vector.tensor_copy(
                        ctxT[:, dm, :, ds(col0, GQ)],
                        pt.rearrange("p (h i) -> p h i", h=H))

            # ---- chunk tail: inner, project ----
            ccol = ds(c * CHUNK, CHUNK)
            ps_in = ps_mid.tile([128, INNER], _F32, tag="psm")
            for h in range(8):
                hs = ds(h * DH, DH)
                for ko in range(4):
                    nc.tensor.matmul(ps_in[:, hs], ctxT[:, ko, h, ccol],
                                     wvs[:, ko, hs],
                                     start=(ko == 0), stop=False)
                for lk in range(2):
                    nc.tensor.matmul(ps_in[:, hs], agT[:, lk, h, ccol],
                                     v_g[:, lk, hs],
                                     start=False, stop=(lk == 1))
            inb = work.tile([128, INNER], _BF16, tag="inb")
            nc.vector.tensor_copy(inb, ps_in)
            if taps is not None:
                nc.gpsimd.dma_start(out=taps["inner"][c], in_=inb)
                nc.gpsimd.dma_start(out=taps["ctxT"][:, :, :, ccol],
                                    in_=ctxT[:, :, :, ccol])
                nc.gpsimd.dma_start(out=taps["agT"][:, :, :, ccol],
                                    in_=agT[:, :, :, ccol])
            itb = work.tile([128, 4, 128], _BF16, tag="itb")
            for io in range(4):
                pt = ps_t.tile([128, 128], _BF16, tag="pst")
                nc.tensor.transpose(pt, inb[:, ts(io, 128)], ident)
                nc.scalar.copy(itb[:, io, :], pt)
            ps_o = ps_mid.tile([128, D], _F32, tag="psm")
            for io in range(4):
                nc.tensor.matmul(ps_o, itb[:, io, :], wout[:, io, :],
                                 start=(io == 0), stop=(io == 3))
            ob = work.tile([128, D], _F32, tag="ob")
            nc.vector.tensor_add(ob, ps_o, bias)
            nc.sync.dma_start(out=out_d[ds(c * CHUNK, CHUNK), :], in_=ob)


def _split_excess_waits(nc, max_waits=1):
    n_split = 0
    for fn in nc.m.functions:
        for blk in fn.blocks:
            out = []
            changed = False
            for inst in blk.instructions:
                si = inst.sync_info
                if si is not None and len(si.on_wait) > max_waits:
                    waits = list(si.on_wait)
                    while len(waits) > max_waits:
                        grp, waits = waits[:max_waits], waits[max_waits:]
                        nop = mybir.InstNoOp(
                            name=f"{inst.name}-wsplit{n_split}", ins=[], outs=[])
                        nop.engine = inst.engine
                        nop.sync_info = mybir.SyncInfo(on_wait=grp, on_update=[])
                        out.append(nop)
                        n_split += 1
                    inst.sync_info = mybir.SyncInfo(
                        on_wait=waits, on_update=list(si.on_update))
                    changed = True
                out.append(inst)
            if changed:
                blk.instructions = out
    return n_split


def _bf16(a):
    return np.ascontiguousarray(a.astype(BF16))


def _shard_inputs(x, spatial_context, global_context, W_q, W_ks, W_vs,
                  W_kg, W_vg, W_out, b_out):
    """Host-side layout prep: shard over (B, N), pre-transpose/cast."""
    pm = lambda a: np.ascontiguousarray(a.reshape(4, 128, -1).transpose(1, 0, 2))
    wq = _bf16(pm(W_q))
    wksT = _bf16(pm(np.ascontiguousarray(W_ks.T)))
    wvs = _bf16(pm(W_vs))
    wkg = _bf16(pm(W_kg))
    wvg = _bf16(pm(W_vg))
    wout = _bf16(pm(W_out))
    biasc = np.ascontiguousarray(b_out.reshape(1, D), dtype=np.float32)

    # off-diagonal spatial mask, rows (h-major (h,i)), cols (i', k)
    maskc = np.zeros((128, TW), dtype=np.float32)
    ii = np.arange(128) % GQ          # i of row (h*16+i)
    jj = np.arange(SW) // K_SP        # i' of col
    maskc[:, :SW] = np.where(ii[:, None] == jj[None, :], 0.0, _NEG)

    in_maps = []
    for c in range(NC):
        b, half = c // 2, c % 2
        sl = slice(half * NQ, (half + 1) * NQ)
        xs = x[b, sl]                                  # (512, 512)
        sp = spatial_context[b, sl].reshape(NQ * K_SP, D)   # (16384, 512)
        gs = global_context[b]                         # (256, 512)

        xT = _bf16(pm(np.ascontiguousarray(xs.T)))
        gT = _bf16(pm(np.ascontiguousarray(gs.T)))
        spn = _bf16(sp.reshape(GROUPS, 4, 128, D).transpose(0, 2, 1, 3))
        spT = np.ascontiguousarray(sp.T)               # (512, 16384)
        spt = _bf16(spT.reshape(4, 128, GROUPS, GROW).transpose(2, 1, 0, 3))

        in_maps.append(dict(
            xT=xT, spn=spn, spt=spt, gT=gT, wq=wq, wksT=wksT, wvs=wvs,
            wkg=wkg, wvg=wvg, wout=wout, maskc=maskc, biasc=biasc))
    return in_maps


_NC_CACHE = None


def _get_module():
    global _NC_CACHE
    if _NC_CACHE is None:
        _NC_CACHE = _build_module()
    return _NC_CACHE


def _install_ntff_shim():
    """Provide antenv.axon_hooks (absent in this image) so
    run_bass_kernel_spmd(trace=True) can capture NTFF profiles through the
    axon PJRT .so, mirroring trn_agent_boot.trn_boot._ntff_profile_via_ctypes."""
    import contextlib
    import ctypes
    import types

    if "antenv.axon_hooks" in sys.modules:
        return
    so_path = "/opt/axon/libaxon_pjrt.so"
    try:
        lib = ctypes.CDLL(so_path)
    except OSError:
        return
    if not hasattr(lib, "axon_start_nrt_profile"):
        return
    lib.axon_start_nrt_profile.argtypes = [ctypes.POINTER(ctypes.c_int64),
                                           ctypes.c_size_t]
    lib.axon_start_nrt_profile.restype = ctypes.c_int64
    lib.axon_stop_nrt_profile.argtypes = [ctypes.c_char_p]
    lib.axon_stop_nrt_profile.restype = ctypes.c_int64

    @contextlib.contextmanager
    def _ctx(output_dir, device_ids):
        import jax
        jax.devices()
        if device_ids:
            ids = (ctypes.c_int64 * len(device_ids))(*device_ids)
            rc = lib.axon_start_nrt_profile(ids, len(device_ids))
        else:
            rc = lib.axon_start_nrt_profile(None, 0)
        if rc != 0:
            raise RuntimeError(f"axon_start_nrt_profile rc={rc}")
        try:
            yield
        finally:
            n = lib.axon_stop_nrt_profile(str(output_dir).encode())
            print(f"ntff: {n} profile file(s) written to {output_dir}")

    mod = types.ModuleType("antenv.axon_hooks")
    mod.get_axon_ntff_profile_hook = lambda: _ctx
    mod.set_axon_ntff_profile_hook = lambda h: None
    sys.modules["antenv.axon_hooks"] = mod


def run_sharded(in_maps, trace=False):
    nc = _get_module()
    if trace:
        _install_ntff_shim()
    return run_bass_kernel_spmd(nc, in_maps, list(range(NC)), trace=trace)


def kernel(x, spatial_context, global_context, W_q, W_ks, W_vs, W_kg,
           W_vg, W_out, b_out, _trace=False, _res_out=None):
    args = [np.asarray(a, dtype=np.float32) for a in
            (x, spatial_context, global_context, W_q, W_ks, W_vs, W_kg,
             W_vg, W_out, b_out)]
    in_maps = _shard_inputs(*args)
    res = run_sharded(in_maps, trace=_trace)
    if _res_out is not None:
        _res_out.append(res)
    full = np.empty((B, N, D), dtype=np.float32)
    for c in range(NC):
        b, half = c // 2, c % 2
        full[b, half * NQ:(half + 1) * NQ] = res.results[c]["out"]
    return full
